# revision 1
# baseline (speedup 1.0000x reference)
"""ChebConv layer (K=3) on 8 TRN2 NeuronCores, data-parallel over batch.

Math:  out = relu(sum_k T_k(L) @ x @ Theta_k),  L = 2A/lambda - I,
       T_0=I, T_1=L, T_2=2L^2-I.
Re-expanded in powers of S = (2/lambda)*A (so no identity terms on device):
       out = relu(Z_A + S @ (Z_B + S @ Z_C))
       Z_C = x@(2*Th2), Z_B = x@(Th1 - 4*Th2), Z_A = x@(Th0 - Th1 + Th2)

Host prep per core (4 batches each):
  st : [4, 1024, 1024] fp8e4m3 = 4096 * S^T per batch (scaled into fp8 range;
                                 the 1/4096 is folded into the PSUM combines)
  xt : [4, 128, 6144]  bf16    = x^T, t-pairs stacked on partitions:
                                 xt[b, (t%2)*64+f, (t//2)*1024+n] = x[b,t,n,f]
  th : [128, 384]      bf16    = [BD(2*Th2) | BD(Th1-4*Th2) | BD(Th0-Th1+Th2)]
                                 BD(M) = blockdiag(M, M) (two t's per matmul)
  out: [4, 8, 128, 768] f32    = per (batch, node-chunk): cols (h, j, t-par, o)

Hop 1 runs as fp8e4m3 DoubleRow matmuls (256-deep contraction per
instruction -> half the matmul count); only Z_C is stored in fp8 -- its
quantization error passes through BOTH strongly-contractive S-aggregations.
Hop 2 stays bf16 (its inputs' errors reach the output through only one
contraction), as do Z_B/Z_A/U.
Emission is software-pipelined (transform of step i+1 before hop1 of step i)
so the PE never waits on PSUM evacuation and stays HAM-warm.
"""

import os
import sys

import numpy as np

sys.path.insert(0, "/opt/trn_rl_repo")

B, T, N, FIN = 32, 12, 1024, 64
K, OUT_F = 3, 64
NCORES = 8
BPC = B // NCORES          # batches per core
NCHUNK = N // 128          # 8 node chunks
TPAIRS = T // 2            # 6
HALVES = 2                 # t-halves; 3 t-pairs each
JW = 3                     # t-pairs per half
SSCALE = 4096.0            # host pre-scale of S into fp8e4m3 normal range

_CACHE = {}
LAST_RESULT = None


def _build_nc():
    import concourse.bacc as bacc
    import concourse.mybir as mybir
    import concourse.tile as tile
    from contextlib import ExitStack

    dt = mybir.dt
    f32, bf16, fp8 = dt.float32, dt.bfloat16, dt.float8e4
    DR = mybir.MatmulPerfMode.DoubleRow

    nc = bacc.Bacc()
    st_d = nc.declare_dram_parameter("st", [BPC, N, N], fp8, isOutput=False)
    stb_d = nc.declare_dram_parameter("stb", [BPC, N, N], bf16, isOutput=False)
    xt_d = nc.declare_dram_parameter("xt", [BPC, 128, TPAIRS * N], bf16, isOutput=False)
    th_d = nc.declare_dram_parameter("th", [128, 3 * 128], bf16, isOutput=False)
    out_d = nc.declare_dram_parameter(
        "out", [BPC, NCHUNK, 128, HALVES * JW * 128], f32, isOutput=True
    )

    with tile.TileContext(nc) as tc, ExitStack() as ctx:
        st_pool = ctx.enter_context(tc.tile_pool(name="stp", bufs=2))
        stb_pool = ctx.enter_context(tc.tile_pool(name="stbp", bufs=2))
        xt_pool = ctx.enter_context(tc.tile_pool(name="xtp", bufs=3))
        th_pool = ctx.enter_context(tc.tile_pool(name="thp", bufs=1))
        zc_pool = ctx.enter_context(tc.tile_pool(name="zcp", bufs=3))
        zba_pool = ctx.enter_context(tc.tile_pool(name="zbap", bufs=3))
        u_pool = ctx.enter_context(tc.tile_pool(name="up", bufs=3))
        o_pool = ctx.enter_context(tc.tile_pool(name="op", bufs=3))
        ps_pool = ctx.enter_context(tc.tile_pool(name="psp", bufs=8, space="PSUM"))

        th_t = th_pool.tile([128, 3 * 128], bf16, name="th_t")
        nc.sync.dma_start(out=th_t[:], in_=th_d[:])

        st_tiles, stb_tiles, xt_tiles, zc_tiles, zba_tiles, u_tiles = {}, {}, {}, {}, {}, {}

        def emit_loads(b):
            xt_t = xt_pool.tile([128, TPAIRS * N], bf16, name=f"xt_{b}", tag="xt")
            nc.sync.dma_start(out=xt_t[:], in_=xt_d[b])
            st_t = st_pool.tile([128, NCHUNK * N], fp8, name=f"st_{b}", tag="st")
            st3 = st_t.rearrange("p (k n) -> p k n", n=N)
            sd3 = st_d[b].rearrange("(k p) n -> p k n", p=128)
            for k in range(0, NCHUNK, 2):
                nc.sync.dma_start(out=st3[:, k : k + 2], in_=sd3[:, k : k + 2])
            stb_t = stb_pool.tile([128, NCHUNK * N], bf16, name=f"stb_{b}", tag="stb")
            stb3 = stb_t.rearrange("p (k n) -> p k n", n=N)
            sb3 = stb_d[b].rearrange("(k p) n -> p k n", p=128)
            for k in range(0, NCHUNK, 2):
                nc.sync.dma_start(out=stb3[:, k : k + 2], in_=sb3[:, k : k + 2])
            st_tiles[b], stb_tiles[b], xt_tiles[b] = st_t, stb_t, xt_t

        def emit_T(b, h):
            if b not in st_tiles:
                emit_loads(b)
            xt_t = xt_tiles[b]
            # zc: per chunk c: cols [c*384, (c+1)*384) = Z_C (j-major), fp8
            # zba: per chunk c: cols [c*768, c*768+384) = Z_B (j-major),
            #                   cols [c*768+384, (c+1)*768) = Z_A (j-major), bf16
            zc = zc_pool.tile(
                [128, NCHUNK * 384], fp8, name=f"zc_{b}_{h}", tag="zc"
            )
            zba = zba_pool.tile(
                [128, NCHUNK * 2 * 384], bf16, name=f"zba_{b}_{h}", tag="zba"
            )
            zc_tiles[(b, h)], zba_tiles[(b, h)] = zc, zba
            for c in range(NCHUNK):
                for j in range(JW):
                    tp = h * JW + j
                    psT = ps_pool.tile([128, 384], f32, name=f"psT_{c}_{j}", tag="ps")
                    nc.tensor.matmul(
                        psT[:],
                        xt_t[:, tp * N + c * 128 : tp * N + (c + 1) * 128],
                        th_t[:],
                        start=True,
                        stop=True,
                    )
                    # psT cols [0:128)=Z_C, [128:256)=Z_B, [256:384)=Z_A
                    nc.vector.tensor_copy(
                        zc[:, c * 384 + j * 128 : c * 384 + (j + 1) * 128],
                        psT[:, 0:128],
                    )
                    ba_dst = zba[
                        :, c * 768 + j * 128 : c * 768 + 384 + (j + 1) * 128
                    ].rearrange("p (s x) -> p s x", x=128)[:, 0::3]
                    nc.scalar.activation(
                        ba_dst,
                        psT[:, 128:384].rearrange("p (s x) -> p s x", x=128),
                        mybir.ActivationFunctionType.Copy,
                    )

        def h1_group(b, h, c):
            st_t, zc_t, zba = st_tiles[b], zc_tiles[(b, h)], zba_tiles[(b, h)]
            st3 = st_t.rearrange("p (k n) -> p k n", n=N)
            zc3 = zc_t.rearrange("p (k r) -> p k r", r=384)
            if c == 0:
                u_tiles[(b, h)] = u_pool.tile(
                    [128, NCHUNK * 384], bf16, name=f"u_{b}_{h}", tag="u"
                )
            u_t = u_tiles[(b, h)]
            ps1 = ps_pool.tile([128, 384], f32, name=f"ps1_{c}", tag="ps")
            for q in range(NCHUNK // 2):
                nc.tensor.matmul(
                    ps1[:],
                    st3[:, 2 * q : 2 * q + 2, c * 128 : (c + 1) * 128],
                    zc3[:, 2 * q : 2 * q + 2, :],
                    start=(q == 0),
                    stop=(q == NCHUNK // 2 - 1),
                    perf_mode=DR,
                )
            nc.vector.scalar_tensor_tensor(
                u_t[:, c * 384 : (c + 1) * 384],
                ps1[:],
                1.0 / SSCALE,
                zba[:, c * 768 : c * 768 + 384],
                op0=mybir.AluOpType.mult,
                op1=mybir.AluOpType.add,
            )

        o_tiles = {}

        def h2_group(b, h, c):
            stb_t, zba, u_t = stb_tiles[b], zba_tiles[(b, h)], u_tiles[(b, h)]
            if c == 0:
                o_tiles[(b, h)] = o_pool.tile(
                    [128, NCHUNK * 384], f32, name=f"o_{b}_{h}", tag="o"
                )
            o_t = o_tiles[(b, h)]
            ps2 = ps_pool.tile([128, 384], f32, name=f"ps2_{c}", tag="ps")
            for k in range(NCHUNK):
                nc.tensor.matmul(
                    ps2[:],
                    stb_t[:, k * N + c * 128 : k * N + (c + 1) * 128],
                    u_t[:, k * 384 : (k + 1) * 384],
                    start=(k == 0),
                    stop=(k == NCHUNK - 1),
                )
            osl = o_t[:, c * 384 : (c + 1) * 384]
            nc.vector.tensor_add(
                osl.rearrange("p (j x) -> p j x", x=128),
                ps2.rearrange("p (j x) -> p j x", x=128),
                zba[:, c * 768 + 384 : (c + 1) * 768].rearrange(
                    "p (j x) -> p j x", x=128
                ),
            )
            nc.scalar.activation(osl, osl, mybir.ActivationFunctionType.Relu)
            nc.sync.dma_start(
                out=out_d[b, c, :, h * 384 : (h + 1) * 384], in_=osl
            )

        # Three-stage skewed pipeline: block i emits T(i+1), then H1(i) groups
        # interleaved 1:1 with H2(i-1) groups so bf16 matmuls keep the PE
        # HAM-warm through the DoubleRow stretches.
        steps = [(b, h) for b in range(BPC) for h in range(HALVES)]
        emit_T(*steps[0])
        emit_T(*steps[1])
        for c in range(NCHUNK):
            h1_group(*steps[0], c)
        for i in range(1, len(steps)):
            if i + 1 < len(steps):
                emit_T(*steps[i + 1])
            for c in range(NCHUNK):
                h1_group(*steps[i], c)
                h2_group(*steps[i - 1], c)
        for c in range(NCHUNK):
            h2_group(*steps[-1], c)
    nc.compile()
    return nc


def _get_nc():
    if "nc" not in _CACHE:
        _CACHE["nc"] = _build_nc()
    return _CACHE["nc"]


def _prep_core(x_c, A_c, TH):
    import ml_dtypes

    lam = np.maximum(A_c.sum(axis=-1).max(axis=-1), 1.0)  # [BPC]
    sT = A_c.transpose(0, 2, 1) * (2.0 / lam)[:, None, None]
    st = np.ascontiguousarray(
        np.clip(sT * SSCALE, 0.0, 240.0).astype(ml_dtypes.float8_e4m3)
    )
    stb = np.ascontiguousarray(sT.astype(ml_dtypes.bfloat16))
    xt = np.ascontiguousarray(
        x_c.reshape(BPC, TPAIRS, 2, N, FIN)
        .transpose(0, 2, 4, 1, 3)
        .reshape(BPC, 128, TPAIRS * N)
        .astype(ml_dtypes.bfloat16)
    )
    return {"st": st, "stb": stb, "xt": xt, "th": TH}


def kernel(x, A, Theta):
    global LAST_RESULT
    import ml_dtypes
    from concourse.bass_utils import run_bass_kernel_spmd

    x = np.asarray(x, dtype=np.float32)
    A = np.asarray(A, dtype=np.float32)
    Theta = np.asarray(Theta, dtype=np.float32)

    T0, T1, T2 = Theta[0], Theta[1], Theta[2]
    folded = [2.0 * T2, T1 - 4.0 * T2, T0 - T1 + T2]
    TH = np.zeros((128, 3 * 128), np.float32)
    for q, M in enumerate(folded):
        TH[0:64, q * 128 : q * 128 + 64] = M
        TH[64:128, q * 128 + 64 : q * 128 + 128] = M
    TH = TH.astype(ml_dtypes.bfloat16)

    nc = _get_nc()
    in_maps = [
        _prep_core(x[c * BPC : (c + 1) * BPC], A[c * BPC : (c + 1) * BPC], TH)
        for c in range(NCORES)
    ]
    trace = bool(int(os.environ.get("CHEB_TRACE", "0")))
    res = run_bass_kernel_spmd(nc, in_maps, list(range(NCORES)), trace=trace)
    LAST_RESULT = res

    outs = []
    for c in range(NCORES):
        od = np.asarray(res.results[c]["out"])  # [BPC, 8, 128, 768]
        r = (
            od.reshape(BPC, NCHUNK, 128, HALVES, JW, 2, OUT_F)
            .transpose(0, 3, 4, 5, 1, 2, 6)
            .reshape(BPC, T, N, OUT_F)
        )
        outs.append(r)
    return np.ascontiguousarray(np.concatenate(outs, axis=0).astype(np.float32))



# revision 9
# speedup vs baseline: 1.0866x; 1.0866x over previous
"""ChebConv layer (K=3) on 8 TRN2 NeuronCores, data-parallel over batch.

Math:  out = relu(sum_k T_k(L) @ x @ Theta_k),  L = 2A/lambda - I,
       T_0=I, T_1=L, T_2=2L^2-I.
Re-expanded in powers of S = (2/lambda)*A (no identity terms on device):
       out = relu(Z_A + S @ (Z_B + S @ Z_C))
       Z_C = x@(2*Th2), Z_B = x@(Th1 - 4*Th2), Z_A = x@(Th0 - Th1 + Th2)

All-fp8-DoubleRow pipeline, layout-parity-clean:
  T_CB : fp8 DR, x-stationary quad-t blockdiag -> Z_C|Z_B normal [n, t*o]
  T_A  : bf16 theta-stationary (one stationary reused) -> Z_A^T [t*o, n]
  H1   : normal,  U[n,to]   = S-pieces (stationary) @ Z_C (moving)  + Z_B
  H2   : transposed, O^T[to,n] = U-pieces (stationary) @ S^T (moving) + Z_A^T
Output DMAed as bf16 O^T pieces; host transposes/upcasts.

Scales: st = S^T*4096 (fp8), xq = x*16 (fp8), thq = theta*64 (fp8),
zcb/u stored *4 (fp8); combines: zcb = psum/256, u = ps1/4096 + zb,
o = ps2/16384 + za.  to-index = t*64+o throughout (t-major).
"""

import os
import sys

import numpy as np

sys.path.insert(0, "/opt/trn_rl_repo")

B, T, N, FIN = 32, 12, 1024, 64
K, OUT_F = 3, 64
NCORES = 8
BPC = B // NCORES          # batches per core
NCHUNK = N // 128          # 8 node chunks
TP = T // 2                # 6 t-pairs (also output to-chunks)
TQ = T // 4                # 3 t-quads
SSCALE = 4096.0            # host pre-scale of S into fp8e4m3 range
XS = 16.0                  # x -> fp8 scale
TS = 64.0                  # theta -> fp8 scale
ZS = 4.0                   # Z_C/Z_B/U fp8 storage scale
FP8MAX = 240.0             # TRN fp8e4 saturation point (beyond -> Inf)

_CACHE = {}
LAST_RESULT = None


def _build_nc():
    import concourse.bacc as bacc
    import concourse.mybir as mybir
    import concourse.tile as tile
    from contextlib import ExitStack

    dt = mybir.dt
    f32, bf16, fp8 = dt.float32, dt.bfloat16, dt.float8e4
    DR = mybir.MatmulPerfMode.DoubleRow
    ACT = mybir.ActivationFunctionType

    nc = bacc.Bacc()
    st_d = nc.declare_dram_parameter("st", [BPC, N, N], fp8, isOutput=False)
    xt_d = nc.declare_dram_parameter("xt", [BPC, 128, TP * N], bf16, isOutput=False)
    xq_d = nc.declare_dram_parameter("xq", [BPC, 128, 2, TQ * N], fp8, isOutput=False)
    thq_d = nc.declare_dram_parameter("thq", [128, 2, 512], fp8, isOutput=False)
    tha_d = nc.declare_dram_parameter("tha", [128, 128], bf16, isOutput=False)
    out_d = nc.declare_dram_parameter(
        "out", [BPC, TP, 128, N], bf16, isOutput=True
    )

    with tile.TileContext(nc) as tc, ExitStack() as ctx:
        st_pool = ctx.enter_context(tc.tile_pool(name="stp", bufs=3))
        xt_pool = ctx.enter_context(tc.tile_pool(name="xtp", bufs=2))
        xq_pool = ctx.enter_context(tc.tile_pool(name="xqp", bufs=2))
        th_pool = ctx.enter_context(tc.tile_pool(name="thp", bufs=1))
        zcb_pool = ctx.enter_context(tc.tile_pool(name="zcbp", bufs=2))
        za_pool = ctx.enter_context(tc.tile_pool(name="zap", bufs=3))
        u_pool = ctx.enter_context(tc.tile_pool(name="up", bufs=2))
        o_pool = ctx.enter_context(tc.tile_pool(name="op", bufs=3))
        ps_pool = ctx.enter_context(tc.tile_pool(name="psp", bufs=2, space="PSUM"))

        thq_t = th_pool.tile([128, 2, 512], fp8, name="thq_t", tag="thq")
        nc.sync.dma_start(out=thq_t[:], in_=thq_d[:])
        tha_t = th_pool.tile([128, 128], bf16, name="tha_t", tag="tha")
        nc.sync.dma_start(out=tha_t[:], in_=tha_d[:])

        st_tiles, xt_tiles, xq_tiles = {}, {}, {}
        zcb_tiles, za_tiles, u_tiles = {}, {}, {}

        def emit_loads(b):
            if b in st_tiles:
                return
            xt_t = xt_pool.tile([128, TP * N], bf16, name=f"xt_{b}", tag="xt")
            nc.sync.dma_start(out=xt_t[:], in_=xt_d[b])
            xq_t = xq_pool.tile([128, 2, TQ * N], fp8, name=f"xq_{b}", tag="xq")
            nc.sync.dma_start(out=xq_t[:], in_=xq_d[b])
            st_t = st_pool.tile([128, NCHUNK * N], fp8, name=f"st_{b}", tag="st")
            st3 = st_t.rearrange("p (k n) -> p k n", n=N)
            sd3 = st_d[b].rearrange("(k p) n -> p k n", p=128)
            for k in range(0, NCHUNK, 2):
                nc.sync.dma_start(out=st3[:, k : k + 2], in_=sd3[:, k : k + 2])
            st_tiles[b], xt_tiles[b], xq_tiles[b] = st_t, xt_t, xq_t

        # ---- transform C/B: psum[n-chunk, (theta, tau*64+o)] per (b,tq,c) ----
        def emit_tcb(b, tq, c):
            xq_t = xq_tiles[b]
            if b not in zcb_tiles:
                zc_t = zcb_pool.tile(
                    [128, NCHUNK, TP * 128], fp8, name=f"zc_{b}", tag="zc"
                )
                zb_t = zcb_pool.tile(
                    [128, NCHUNK, TP * 128], bf16, name=f"zb_{b}", tag="zb"
                )
                zcb_tiles[b] = (zc_t, zb_t)
            zc_t, zb_t = zcb_tiles[b]
            psT = ps_pool.tile([128, 512], f32, name=f"psT_{b}_{tq}_{c}", tag="tr")
            nc.tensor.matmul(
                psT[:],
                xq_t[:, :, tq * N + c * 128 : tq * N + (c + 1) * 128],
                thq_t[:],
                start=True,
                stop=True,
                perf_mode=DR,
            )
            # psT cols: 0:256 -> Z_C*1024, 256:512 -> Z_B*1024 (tau-major)
            # zb stays bf16: U ~= Z_B + small S@Z_C, so an fp8 zb makes the
            # later fp8 U write a double-rounding that mangles the increment.
            dzc = zc_t[:, c, tq * 256 : (tq + 1) * 256]
            dzb = zb_t[:, c, tq * 256 : (tq + 1) * 256]
            if (tq * NCHUNK + c) % 2 == 0:
                nc.vector.tensor_scalar_mul(dzc, psT[:, 0:256], 1.0 / 256.0)
                nc.scalar.activation(dzb, psT[:, 256:512], ACT.Copy, scale=1.0 / 256.0)
            else:
                nc.scalar.activation(dzc, psT[:, 0:256], ACT.Copy, scale=1.0 / 256.0)
                nc.vector.tensor_scalar_mul(dzb, psT[:, 256:512], 1.0 / 256.0)

        # ---- transform A: psum[(par,o), n-half] per (b,tp,h): Z_A^T ----
        def emit_ta(b, tp, h):
            xt_t = xt_tiles[b]
            if b not in za_tiles:
                za_tiles[b] = za_pool.tile(
                    [128, TP, N], bf16, name=f"za_{b}", tag="za"
                )
            za = za_tiles[b]
            psA = ps_pool.tile([128, 512], f32, name=f"psA_{b}_{tp}_{h}", tag="tr")
            nc.tensor.matmul(
                psA[:],
                tha_t[:],
                xt_t[:, tp * N + h * 512 : tp * N + (h + 1) * 512],
                start=True,
                stop=True,
            )
            nc.scalar.activation(
                za[:, tp, h * 512 : (h + 1) * 512], psA[:], ACT.Copy
            )

        # ---- hop1 (normal): U[n-chunk c, to] = S@Z_C + Z_B ----
        def h1_group(b, c):
            st3 = st_tiles[b].rearrange("p (k n) -> p k n", n=N)
            zc_t, zb_t = zcb_tiles[b]
            if b not in u_tiles:
                u_tiles[b] = u_pool.tile(
                    [128, NCHUNK, TP * 128], fp8, name=f"u_{b}", tag="u"
                )
            u3 = u_tiles[b]
            ps1 = ps_pool.tile([128, 768], f32, name=f"ps1_{b}_{c}", tag="ps1")
            for q in range(NCHUNK // 2):
                lw = st3[:, 2 * q : 2 * q + 2, c * 128 : (c + 1) * 128]
                nc.tensor.matmul(
                    ps1[:, 0:512],
                    lw,
                    zc_t[:, 2 * q : 2 * q + 2, 0:512],
                    start=(q == 0),
                    stop=(q == NCHUNK // 2 - 1),
                    perf_mode=DR,
                )
                nc.tensor.matmul(
                    ps1[:, 512:768],
                    lw,
                    zc_t[:, 2 * q : 2 * q + 2, 512:768],
                    start=(q == 0),
                    stop=(q == NCHUNK // 2 - 1),
                    perf_mode=DR,
                )
            nc.vector.scalar_tensor_tensor(
                u3[:, c, :],
                ps1[:],
                1.0 / 4096.0,
                zb_t[:, c, :],
                op0=mybir.AluOpType.mult,
                op1=mybir.AluOpType.add,
            )

        # ---- hop2 (transposed): O^T[to-chunk j, n] = U^T@S^T + Z_A^T ----
        def h2_group(b, j):
            st3 = st_tiles[b].rearrange("p (k n) -> p k n", n=N)
            u3, za = u_tiles[b], za_tiles[b]
            ps2 = ps_pool.tile([128, N], f32, name=f"ps2_{b}_{j}", tag="ps2", bufs=1)
            for q in range(NCHUNK // 2):
                lw = u3[:, 2 * q : 2 * q + 2, j * 128 : (j + 1) * 128]
                for h in range(2):
                    nc.tensor.matmul(
                        ps2[:, h * 512 : (h + 1) * 512],
                        lw,
                        st3[:, 2 * q : 2 * q + 2, h * 512 : (h + 1) * 512],
                        start=(q == 0),
                        stop=(q == NCHUNK // 2 - 1),
                        perf_mode=DR,
                    )
            o_t = o_pool.tile([128, N], bf16, name=f"o_{b}_{j}", tag="o")
            nc.vector.scalar_tensor_tensor(
                o_t[:],
                ps2[:],
                1.0 / 16384.0,
                za[:, j, :],
                op0=mybir.AluOpType.mult,
                op1=mybir.AluOpType.add,
            )
            nc.scalar.activation(o_t[:], o_t[:], ACT.Relu)
            nc.sync.dma_start(out=out_d[b, j], in_=o_t[:])

        # ---- transform emission for one batch, interleavable in slices ----
        def t_units(b):
            units = []
            for tq in range(TQ):
                for c in range(NCHUNK):
                    units.append(("cb", b, tq, c))
            for tp in range(TP):
                for h in range(2):
                    units.append(("a", b, tp, h))
            return units

        def run_units(units):
            for u in units:
                if u[0] == "cb":
                    emit_tcb(*u[1:])
                else:
                    emit_ta(*u[1:])

        # ---- software pipeline over batches ----
        # step b: emit T(b+1) units interleaved with H1(b) and H2(b-1) groups
        emit_loads(0)
        emit_loads(1)
        run_units(t_units(0))
        for b in range(BPC):
            if b + 1 < BPC:
                emit_loads(b + 1)
                units = t_units(b + 1)
            else:
                units = []
            # 8 H1 groups for b, 6 H2 groups for b-1, 36 transform units
            nslots = NCHUNK
            per = (len(units) + nslots - 1) // nslots if units else 0
            for c in range(NCHUNK):
                run_units(units[c * per : (c + 1) * per])
                h1_group(b, c)
                if b > 0 and c < TP:
                    h2_group(b - 1, c)
        for j in range(TP):
            h2_group(BPC - 1, j)
    nc.compile()
    return nc


def _get_nc():
    if "nc" not in _CACHE:
        _CACHE["nc"] = _build_nc()
    return _CACHE["nc"]


def _to_fp8(a):
    import ml_dtypes

    return np.clip(a, -FP8MAX, FP8MAX).astype(ml_dtypes.float8_e4m3)


def _prep_core(x_c, A_c, THQ, THA):
    import ml_dtypes

    lam = np.maximum(A_c.sum(axis=-1).max(axis=-1), 1.0)  # [BPC]
    sT = A_c.transpose(0, 2, 1) * (2.0 / lam)[:, None, None]
    st = np.ascontiguousarray(_to_fp8(sT * SSCALE))
    # xt[b, par*64+f, tp*N+n] = x[b, 2tp+par, n, f]
    xt = np.ascontiguousarray(
        x_c.reshape(BPC, TP, 2, N, FIN)
        .transpose(0, 2, 4, 1, 3)
        .reshape(BPC, 128, TP * N)
        .astype(ml_dtypes.bfloat16)
    )
    # xq[b, par*64+f, pk, tq*N+n] = x[b, 4tq+2pk+par, n, f] * XS
    xq = np.ascontiguousarray(
        _to_fp8(
            x_c.reshape(BPC, TQ, 2, 2, N, FIN)  # b, tq, pk, par, n, f
            .transpose(0, 3, 5, 2, 1, 4)        # b, par, f, pk, tq, n
            .reshape(BPC, 128, 2, TQ * N)
            * XS
        )
    )
    return {"st": st, "xt": xt, "xq": xq, "thq": THQ, "tha": THA}


def kernel(x, A, Theta):
    global LAST_RESULT
    import ml_dtypes
    from concourse.bass_utils import run_bass_kernel_spmd

    x = np.asarray(x, dtype=np.float32)
    A = np.asarray(A, dtype=np.float32)
    Theta = np.asarray(Theta, dtype=np.float32)

    T0, T1, T2 = Theta[0], Theta[1], Theta[2]
    thC, thB, thA = 2.0 * T2, T1 - 4.0 * T2, T0 - T1 + T2

    # thq[par*64+f, pk, s*256 + (2pk+par)*64 + o] = th_s[f, o] * TS
    THQ = np.zeros((128, 2, 512), np.float32)
    for s, M in enumerate([thC, thB]):
        for pk in range(2):
            for par in range(2):
                tau = 2 * pk + par
                THQ[par * 64 : par * 64 + 64, pk,
                    s * 256 + tau * 64 : s * 256 + tau * 64 + 64] = M * TS
    THQ = _to_fp8(THQ)

    # tha[par*64+f, par*64+o] = thA[f, o]  (pair blockdiag)
    THA = np.zeros((128, 128), np.float32)
    THA[0:64, 0:64] = thA
    THA[64:128, 64:128] = thA
    THA = THA.astype(ml_dtypes.bfloat16)

    nc = _get_nc()
    in_maps = [
        _prep_core(x[c * BPC : (c + 1) * BPC], A[c * BPC : (c + 1) * BPC],
                   THQ, THA)
        for c in range(NCORES)
    ]
    trace = bool(int(os.environ.get("CHEB_TRACE", "0")))
    res = run_bass_kernel_spmd(nc, in_maps, list(range(NCORES)), trace=trace)
    LAST_RESULT = res

    outs = []
    for c in range(NCORES):
        od = np.asarray(res.results[c]["out"])  # [BPC, 6, 128, 1024] bf16
        # od[b, j, par*64+o, n] = out[b, 2j+par, n, o]
        r = (
            od.astype(np.float32)
            .reshape(BPC, TP, 2, OUT_F, N)   # b, j, par, o, n
            .transpose(0, 1, 2, 4, 3)        # b, j, par, n, o
            .reshape(BPC, T, N, OUT_F)
        )
        outs.append(r)
    return np.ascontiguousarray(np.concatenate(outs, axis=0).astype(np.float32))


# revision 12
# speedup vs baseline: 1.2803x; 1.1783x over previous
"""ChebConv layer (K=3) on 8 TRN2 NeuronCores, data-parallel over batch.

Math:  out = relu(sum_k T_k(L) @ x @ Theta_k),  L = 2A/lambda - I,
       T_0=I, T_1=L, T_2=2L^2-I.
Re-expanded in powers of S = (2/lambda)*A (no identity terms on device):
       out = relu(Z_A + S @ (Z_B + S @ Z_C))
       Z_C = x@(2*Th2), Z_B = x@(Th1 - 4*Th2), Z_A = x@(Th0 - Th1 + Th2)

All-fp8-DoubleRow pipeline, layout-parity-clean:
  T_CB : fp8 DR, x-stationary quad-t blockdiag -> Z_C|Z_B normal [n, t*o]
  T_A  : bf16 theta-stationary (one stationary reused) -> Z_A^T [t*o, n]
  H1   : normal,  U[n,to]   = S-pieces (stationary) @ Z_C (moving)  + Z_B
  H2   : transposed, O^T[to,n] = U-pieces (stationary) @ S^T (moving) + Z_A^T
Output DMAed as bf16 O^T pieces; host transposes/upcasts.

Scales: st = S^T*4096 (fp8), xq = x*16 (fp8), thq = theta*64 (fp8),
zcb/u stored *4 (fp8); combines: zcb = psum/256, u = ps1/4096 + zb,
o = ps2/16384 + za.  to-index = t*64+o throughout (t-major).
"""

import os
import sys

import numpy as np

sys.path.insert(0, "/opt/trn_rl_repo")

B, T, N, FIN = 32, 12, 1024, 64
K, OUT_F = 3, 64
NCORES = 8
BPC = B // NCORES          # batches per core
NCHUNK = N // 128          # 8 node chunks
TP = T // 2                # 6 t-pairs (also output to-chunks)
TQ = T // 4                # 3 t-quads
SSCALE = 4096.0            # host pre-scale of S into fp8e4m3 range
XS = 16.0                  # x -> fp8 scale
TS = 64.0                  # theta -> fp8 scale
ZS = 4.0                   # Z_C/Z_B/U fp8 storage scale
FP8MAX = 240.0             # TRN fp8e4 saturation point (beyond -> Inf)

_CACHE = {}
LAST_RESULT = None


def _build_nc():
    import concourse.bacc as bacc
    import concourse.mybir as mybir
    import concourse.tile as tile
    from contextlib import ExitStack

    dt = mybir.dt
    f32, bf16, fp8 = dt.float32, dt.bfloat16, dt.float8e4
    DR = mybir.MatmulPerfMode.DoubleRow
    ACT = mybir.ActivationFunctionType

    nc = bacc.Bacc()
    st_d = nc.declare_dram_parameter("st", [BPC, N, N], fp8, isOutput=False)
    xt_d = nc.declare_dram_parameter("xt", [BPC, 128, TP * N], bf16, isOutput=False)
    xq_d = nc.declare_dram_parameter("xq", [BPC, 128, 2, TQ * N], fp8, isOutput=False)
    thq_d = nc.declare_dram_parameter("thq", [128, 2, 512], fp8, isOutput=False)
    tha_d = nc.declare_dram_parameter("tha", [128, 128], bf16, isOutput=False)
    out_d = nc.declare_dram_parameter(
        "out", [BPC, TP, 128, N], bf16, isOutput=True
    )

    with tile.TileContext(nc) as tc, ExitStack() as ctx:
        st_pool = ctx.enter_context(tc.tile_pool(name="stp", bufs=3))
        xt_pool = ctx.enter_context(tc.tile_pool(name="xtp", bufs=2))
        xq_pool = ctx.enter_context(tc.tile_pool(name="xqp", bufs=2))
        th_pool = ctx.enter_context(tc.tile_pool(name="thp", bufs=1))
        zcb_pool = ctx.enter_context(tc.tile_pool(name="zcbp", bufs=2))
        za_pool = ctx.enter_context(tc.tile_pool(name="zap", bufs=3))
        u_pool = ctx.enter_context(tc.tile_pool(name="up", bufs=2))
        o_pool = ctx.enter_context(tc.tile_pool(name="op", bufs=3))
        ps_pool = ctx.enter_context(tc.tile_pool(name="psp", bufs=2, space="PSUM"))

        thq_t = th_pool.tile([128, 2, 512], fp8, name="thq_t", tag="thq")
        nc.sync.dma_start(out=thq_t[:], in_=thq_d[:])
        tha_t = th_pool.tile([128, 128], bf16, name="tha_t", tag="tha")
        nc.sync.dma_start(out=tha_t[:], in_=tha_d[:])

        st_tiles, xt_tiles, xq_tiles = {}, {}, {}
        zcb_tiles, za_tiles, u_tiles = {}, {}, {}

        def emit_loads(b):
            if b in st_tiles:
                return
            xt_t = xt_pool.tile([128, TP * N], bf16, name=f"xt_{b}", tag="xt")
            nc.sync.dma_start(out=xt_t[:], in_=xt_d[b])
            xq_t = xq_pool.tile([128, 2, TQ * N], fp8, name=f"xq_{b}", tag="xq")
            nc.sync.dma_start(out=xq_t[:], in_=xq_d[b])
            st_t = st_pool.tile([128, NCHUNK * N], fp8, name=f"st_{b}", tag="st")
            st3 = st_t.rearrange("p (k n) -> p k n", n=N)
            sd3 = st_d[b].rearrange("(k p) n -> p k n", p=128)
            for k in range(0, NCHUNK, 2):
                nc.sync.dma_start(out=st3[:, k : k + 2], in_=sd3[:, k : k + 2])
            st_tiles[b], xt_tiles[b], xq_tiles[b] = st_t, xt_t, xq_t

        # ---- transform C/B: psum[n-chunk, (theta, tau*64+o)] per (b,tq,c) ----
        def emit_tcb(b, tq, c):
            xq_t = xq_tiles[b]
            if b not in zcb_tiles:
                zc_t = zcb_pool.tile(
                    [128, NCHUNK, TP * 128], fp8, name=f"zc_{b}", tag="zc"
                )
                zb_t = zcb_pool.tile(
                    [128, NCHUNK, TP * 128], bf16, name=f"zb_{b}", tag="zb"
                )
                zcb_tiles[b] = (zc_t, zb_t)
            zc_t, zb_t = zcb_tiles[b]
            psT = ps_pool.tile([128, 512], f32, name=f"psT_{b}_{tq}_{c}", tag="tr")
            nc.tensor.matmul(
                psT[:],
                xq_t[:, :, tq * N + c * 128 : tq * N + (c + 1) * 128],
                thq_t[:],
                start=True,
                stop=True,
                perf_mode=DR,
            )
            # psT cols: 0:256 -> Z_C*1024, 256:512 -> Z_B*1024 (tau-major)
            # zb stays bf16: U ~= Z_B + small S@Z_C, so an fp8 zb makes the
            # later fp8 U write a double-rounding that mangles the increment.
            dzc = zc_t[:, c, tq * 256 : (tq + 1) * 256]
            dzb = zb_t[:, c, tq * 256 : (tq + 1) * 256]
            if (tq * NCHUNK + c) % 2 == 0:
                nc.vector.tensor_scalar_mul(dzc, psT[:, 0:256], 1.0 / 256.0)
                nc.scalar.activation(dzb, psT[:, 256:512], ACT.Copy, scale=1.0 / 256.0)
            else:
                nc.scalar.activation(dzc, psT[:, 0:256], ACT.Copy, scale=1.0 / 256.0)
                nc.vector.tensor_scalar_mul(dzb, psT[:, 256:512], 1.0 / 256.0)

        # ---- transform A: psum[(par,o), n-half] per (b,tp,h): Z_A^T ----
        def emit_ta(b, tp, h):
            xt_t = xt_tiles[b]
            if b not in za_tiles:
                za_tiles[b] = za_pool.tile(
                    [128, TP, N], bf16, name=f"za_{b}", tag="za"
                )
            za = za_tiles[b]
            psA = ps_pool.tile([128, 512], f32, name=f"psA_{b}_{tp}_{h}", tag="tr")
            nc.tensor.matmul(
                psA[:],
                tha_t[:],
                xt_t[:, tp * N + h * 512 : tp * N + (h + 1) * 512],
                start=True,
                stop=True,
            )
            nc.scalar.activation(
                za[:, tp, h * 512 : (h + 1) * 512], psA[:], ACT.Copy
            )

        # ---- hop1 (normal): U[n-chunk c, to] = S@Z_C + Z_B ----
        def h1_group(b, c):
            st3 = st_tiles[b].rearrange("p (k n) -> p k n", n=N)
            zc_t, zb_t = zcb_tiles[b]
            if b not in u_tiles:
                u_tiles[b] = u_pool.tile(
                    [128, NCHUNK, TP * 128], fp8, name=f"u_{b}", tag="u"
                )
            u3 = u_tiles[b]
            p1a = ps_pool.tile([128, 512], f32, name=f"p1a_{b}_{c}", tag="p1a")
            p1b = ps_pool.tile([128, 256], f32, name=f"p1b_{b}_{c}", tag="p1b")
            for q in range(NCHUNK // 2):
                lw = st3[:, 2 * q : 2 * q + 2, c * 128 : (c + 1) * 128]
                nc.tensor.matmul(
                    p1a[:],
                    lw,
                    zc_t[:, 2 * q : 2 * q + 2, 0:512],
                    start=(q == 0),
                    stop=(q == NCHUNK // 2 - 1),
                    perf_mode=DR,
                )
                nc.tensor.matmul(
                    p1b[:],
                    lw,
                    zc_t[:, 2 * q : 2 * q + 2, 512:768],
                    start=(q == 0),
                    stop=(q == NCHUNK // 2 - 1),
                    perf_mode=DR,
                )
            nc.vector.scalar_tensor_tensor(
                u3[:, c, 0:512],
                p1a[:],
                1.0 / 4096.0,
                zb_t[:, c, 0:512],
                op0=mybir.AluOpType.mult,
                op1=mybir.AluOpType.add,
            )
            nc.vector.scalar_tensor_tensor(
                u3[:, c, 512:768],
                p1b[:],
                1.0 / 4096.0,
                zb_t[:, c, 512:768],
                op0=mybir.AluOpType.mult,
                op1=mybir.AluOpType.add,
            )

        # ---- hop2 (transposed): O^T[to-chunk j, n] = U^T@S^T + Z_A^T ----
        def h2_group(b, j):
            st3 = st_tiles[b].rearrange("p (k n) -> p k n", n=N)
            u3, za = u_tiles[b], za_tiles[b]
            o_t = o_pool.tile([128, N], bf16, name=f"o_{b}_{j}", tag="o")
            for h in range(2):
                ps2 = ps_pool.tile(
                    [128, 512], f32, name=f"ps2_{b}_{j}_{h}", tag="ps2"
                )
                for q in range(NCHUNK // 2):
                    nc.tensor.matmul(
                        ps2[:],
                        u3[:, 2 * q : 2 * q + 2, j * 128 : (j + 1) * 128],
                        st3[:, 2 * q : 2 * q + 2, h * 512 : (h + 1) * 512],
                        start=(q == 0),
                        stop=(q == NCHUNK // 2 - 1),
                        perf_mode=DR,
                    )
                nc.vector.scalar_tensor_tensor(
                    o_t[:, h * 512 : (h + 1) * 512],
                    ps2[:],
                    1.0 / 16384.0,
                    za[:, j, h * 512 : (h + 1) * 512],
                    op0=mybir.AluOpType.mult,
                    op1=mybir.AluOpType.add,
                )
            nc.scalar.activation(o_t[:], o_t[:], ACT.Relu)
            nc.sync.dma_start(out=out_d[b, j], in_=o_t[:])

        # ---- transform emission for one batch, interleavable in slices ----
        def t_units(b):
            units = []
            for tq in range(TQ):
                for c in range(NCHUNK):
                    units.append(("cb", b, tq, c))
            for tp in range(TP):
                for h in range(2):
                    units.append(("a", b, tp, h))
            return units

        def run_units(units):
            for u in units:
                if u[0] == "cb":
                    emit_tcb(*u[1:])
                else:
                    emit_ta(*u[1:])

        # ---- software pipeline over batches ----
        # step b: emit T(b+1) units interleaved with H1(b) and H2(b-1) groups
        emit_loads(0)
        emit_loads(1)
        run_units(t_units(0))
        for b in range(BPC):
            if b + 1 < BPC:
                emit_loads(b + 1)
                units = t_units(b + 1)
            else:
                units = []
            # 8 H1 groups for b, 6 H2 groups for b-1, 36 transform units
            nslots = NCHUNK
            per = (len(units) + nslots - 1) // nslots if units else 0
            for c in range(NCHUNK):
                run_units(units[c * per : (c + 1) * per])
                if b > 0 and c < TP:
                    h2_group(b - 1, c)
                h1_group(b, c)
        for j in range(TP):
            h2_group(BPC - 1, j)
    nc.compile()
    return nc


def _get_nc():
    if "nc" not in _CACHE:
        _CACHE["nc"] = _build_nc()
    return _CACHE["nc"]


def _to_fp8(a):
    import ml_dtypes

    return np.clip(a, -FP8MAX, FP8MAX).astype(ml_dtypes.float8_e4m3)


def _prep_core(x_c, A_c, THQ, THA):
    import ml_dtypes

    lam = np.maximum(A_c.sum(axis=-1).max(axis=-1), 1.0)  # [BPC]
    sT = A_c.transpose(0, 2, 1) * (2.0 / lam)[:, None, None]
    st = np.ascontiguousarray(_to_fp8(sT * SSCALE))
    # xt[b, par*64+f, tp*N+n] = x[b, 2tp+par, n, f]
    xt = np.ascontiguousarray(
        x_c.reshape(BPC, TP, 2, N, FIN)
        .transpose(0, 2, 4, 1, 3)
        .reshape(BPC, 128, TP * N)
        .astype(ml_dtypes.bfloat16)
    )
    # xq[b, par*64+f, pk, tq*N+n] = x[b, 4tq+2pk+par, n, f] * XS
    xq = np.ascontiguousarray(
        _to_fp8(
            x_c.reshape(BPC, TQ, 2, 2, N, FIN)  # b, tq, pk, par, n, f
            .transpose(0, 3, 5, 2, 1, 4)        # b, par, f, pk, tq, n
            .reshape(BPC, 128, 2, TQ * N)
            * XS
        )
    )
    return {"st": st, "xt": xt, "xq": xq, "thq": THQ, "tha": THA}


def kernel(x, A, Theta):
    global LAST_RESULT
    import ml_dtypes
    from concourse.bass_utils import run_bass_kernel_spmd

    x = np.asarray(x, dtype=np.float32)
    A = np.asarray(A, dtype=np.float32)
    Theta = np.asarray(Theta, dtype=np.float32)

    T0, T1, T2 = Theta[0], Theta[1], Theta[2]
    thC, thB, thA = 2.0 * T2, T1 - 4.0 * T2, T0 - T1 + T2

    # thq[par*64+f, pk, s*256 + (2pk+par)*64 + o] = th_s[f, o] * TS
    THQ = np.zeros((128, 2, 512), np.float32)
    for s, M in enumerate([thC, thB]):
        for pk in range(2):
            for par in range(2):
                tau = 2 * pk + par
                THQ[par * 64 : par * 64 + 64, pk,
                    s * 256 + tau * 64 : s * 256 + tau * 64 + 64] = M * TS
    THQ = _to_fp8(THQ)

    # tha[par*64+f, par*64+o] = thA[f, o]  (pair blockdiag)
    THA = np.zeros((128, 128), np.float32)
    THA[0:64, 0:64] = thA
    THA[64:128, 64:128] = thA
    THA = THA.astype(ml_dtypes.bfloat16)

    nc = _get_nc()
    in_maps = [
        _prep_core(x[c * BPC : (c + 1) * BPC], A[c * BPC : (c + 1) * BPC],
                   THQ, THA)
        for c in range(NCORES)
    ]
    trace = bool(int(os.environ.get("CHEB_TRACE", "0")))
    res = run_bass_kernel_spmd(nc, in_maps, list(range(NCORES)), trace=trace)
    LAST_RESULT = res

    outs = []
    for c in range(NCORES):
        od = np.asarray(res.results[c]["out"])  # [BPC, 6, 128, 1024] bf16
        # od[b, j, par*64+o, n] = out[b, 2j+par, n, o]
        r = (
            od.astype(np.float32)
            .reshape(BPC, TP, 2, OUT_F, N)   # b, j, par, o, n
            .transpose(0, 1, 2, 4, 3)        # b, j, par, n, o
            .reshape(BPC, T, N, OUT_F)
        )
        outs.append(r)
    return np.ascontiguousarray(np.concatenate(outs, axis=0).astype(np.float32))
